# revision 9
# baseline (speedup 1.0000x reference)
"""DeepSeekMoE layer on 8 Trainium2 NeuronCores.

Strategy (expert-parallel, matching the sharding hint):
  - Host computes the (tiny) gate: softmax(x @ gate_w + gate_b), top-2
    routing, and per-expert token gather with capacity padding.  This is
    the control plane (<1% of FLOPs); all heavy matmuls run on device.
  - Experts are assigned to cores by LPT (longest-processing-time) over
    the routed token counts; each core's 4 experts are sorted by count so
    slot k holds its k-th largest.  Slot capacities are the per-rank max
    over cores (rounded to 32), which keeps padding minimal even when
    routing is skewed.
  - The 2 shared experts are data-parallel over tokens: core c processes
    tokens [c*512, (c+1)*512) as two 256-token blocks.
  - Both matmuls run with tokens as the MOVING operand so PE cost scales
    with the exact block capacity (no 128-row PSUM granularity waste):
      mm1: hT = w1^T @ xT (K=D in PSUM) -> Gelu(+b1 via ACT bias port),
           g stays [FD-slice, tokens] in SBUF, bf16.
      mm2: per 128-col D-chunk, stationary w2 [FD-slice, 128] tiles
           (streamed as [128, 1280] quarter tiles), moving g -> PSUM
           [128 D-rows, tokens]; evicted bf16 into 4-chunk staging tiles
           and drained D-major.  The top-k combine weight is applied on
           the host in fp32 during the scatter-add (it is elementwise,
           0.04% of FLOPs).
  - Host transposes the D-major outputs and scatter-adds routed + shared
    contributions back into token order.
  - DMA queue discipline (avoids head-of-line blocking between pool-ring
    waits and output drains): sync = w1/x/sw1 loads + all drains;
    scalar = xs loads + gelu + 1/3 of mm2 evictions; gpsimd = w2/sw2
    quarter loads only; vector = 2/3 evictions + bias loads.
All matmul inputs are bf16 (PSUM accumulates fp32); b1 is applied in
fp32 via the ACT bias port; b2/gate_b host-side (zero-guarded).
"""

import os
import sys
import types

import numpy as np

# ---------------------------------------------------------------------------
# Optional NTFF trace support under axon: concourse's trace path imports
# antenv.axon_hooks, which this image lacks; shim it with the boot helper.
# ---------------------------------------------------------------------------
def _install_trace_shim():
    try:
        if "antenv.axon_hooks" in sys.modules:
            return
        from trn_agent_boot.trn_boot import _ntff_profile_via_ctypes

        hook = _ntff_profile_via_ctypes("/opt/axon/libaxon_pjrt.so")
        mod = types.ModuleType("antenv.axon_hooks")
        mod.get_axon_ntff_profile_hook = lambda: hook
        mod.set_axon_ntff_profile_hook = lambda h: None
        sys.modules["antenv.axon_hooks"] = mod
    except Exception:
        pass


_install_trace_shim()

import ml_dtypes  # noqa: E402

import concourse.bacc as bacc  # noqa: E402
import concourse.mybir as mybir  # noqa: E402
import concourse.tile as tile  # noqa: E402
import concourse.bass_utils as bass_utils  # noqa: E402
from concourse.bass import ts, ds  # noqa: E402
from concourse.bass_utils import run_bass_kernel_spmd  # noqa: E402

try:  # zero-egress sandbox: skip artifact uploads during tracing
    bass_utils.upload_artifacts = lambda tmpdir: tmpdir
except Exception:
    pass

# Problem shapes (nn_DeepSeekMoE): x [B,T,D]; E routed experts (top-K),
# S shared experts, ffn dim FD.
B, T, D = 2, 2048, 5120
FD, E, S, K = 384, 32, 2, 2
N = B * T                     # 4096 tokens
P = 128
NCORES = 8
EPC = E // NCORES             # 4 routed experts per core
NS = N // NCORES              # 512 shared-expert tokens per core
NSH = NS // 2                 # shared-block token count (256)
KD = D // P                   # 40 K-subtiles over D
KDH = KD // 2                 # half-tile K depth (20)
KF = FD // P                  # 3 K-subtiles over FD
NDCK = D // P                 # 40 output D-chunks of 128
NG = NDCK // 4                # 10 drain groups of 4 chunks
DQ = D // 4                   # w2 quarter-tile width (1280)

BF16 = ml_dtypes.bfloat16

LAST_EXEC_NS = None
LAST_MEAN_EXEC_NS = None
LAST_TRACE = None


def _ceil_div(a, b):
    return -(-a // b)


def _build_nc(caps, nb, out_dt=mybir.dt.bfloat16):
    """Build the SPMD per-core Bass program.

    caps: per-slot token capacities (each a multiple of 32, <= 512).
    nb:   sub-blocks per expert (slot k belongs to expert k // nb).

    DRAM layouts (pre-swizzled on host so every DMA is contiguous):
      xe{k}  [P, KD, caps[k]]   slot-k tokens, transposed, D-slice-major
      w1p    [EPC, P, KD, FD]   routed w1, p-major over D (rank order)
      w2p    [EPC, KF, P, D]    routed w2, p-major over FD
      xsp    [2, P, KD, NSH]    shared tokens, 2 blocks, transposed
      sw1p   [S, P, KD, FD]     shared w1
      sw2p   [S, KF, P, D]      shared w2
    Outputs are D-major (host transposes):
      yt{k}  [NG, P, 4, caps[k]],  yst [2, NG, P, 4, NSH]
    """
    f32 = mybir.dt.float32
    bf16 = mybir.dt.bfloat16
    nc = bacc.Bacc(None, target_bir_lowering=False)

    nslot = len(caps)
    xe = [
        nc.dram_tensor(f"xe{k}", (P, KD, caps[k]), bf16, kind="ExternalInput")
        for k in range(nslot)
    ]
    yt = [
        nc.dram_tensor(f"yt{k}", (NG, P, 4, caps[k]), out_dt, kind="ExternalOutput")
        for k in range(nslot)
    ]
    w1p = nc.dram_tensor("w1p", (EPC, P, KD, FD), bf16, kind="ExternalInput")
    rb1 = nc.dram_tensor("rb1", (P, EPC * KF), f32, kind="ExternalInput")
    w2p = nc.dram_tensor("w2p", (EPC, KF, P, D), bf16, kind="ExternalInput")
    xsp = nc.dram_tensor("xsp", (2, P, KD, NSH), bf16, kind="ExternalInput")
    sw1p = nc.dram_tensor("sw1p", (S, P, KD, FD), bf16, kind="ExternalInput")
    sb1 = nc.dram_tensor("sb1", (P, S * KF), f32, kind="ExternalInput")
    sw2p = nc.dram_tensor("sw2p", (S, KF, P, D), bf16, kind="ExternalInput")
    yst = nc.dram_tensor("yst", (2, NG, P, 4, NSH), out_dt, kind="ExternalOutput")
    thr = nc.dram_tensor("thr", (8, 1), bf16, kind="Internal")

    gelu = mybir.ActivationFunctionType.Gelu
    copyf = mybir.ActivationFunctionType.Copy

    with tile.TileContext(nc) as tc:
        with (
            tc.tile_pool(name="pw1", bufs=4) as pw1,
            tc.tile_pool(name="pxe", bufs=4) as pxe,
            tc.tile_pool(name="pw2", bufs=24) as pw2,
            tc.tile_pool(name="pgs", bufs=4) as pgs,
            tc.tile_pool(name="pgr", bufs=3) as pgr,
            tc.tile_pool(name="pys", bufs=6) as pys,
            tc.tile_pool(name="pb", bufs=1) as pb,
            tc.tile_pool(name="pps1", bufs=3, space="PSUM") as pps1,
            tc.tile_pool(name="pps2", bufs=4, space="PSUM") as pps2,
        ):
            rb1_sb = pb.tile([P, EPC * KF], f32, tag="rb1")
            sb1_sb = pb.tile([P, S * KF], f32, tag="sb1")

            def emit_biases():
                nc.gpsimd.dma_start(rb1_sb, rb1[:])
                nc.gpsimd.dma_start(sb1_sb, sb1[:])

            def load_halves(pool, src_ap, width, tag, eng, npieces=2):
                """Two [P, KDH, width] tiles for a [P, KD, width] DRAM src."""
                tiles = []
                for h in range(2):
                    t = pool.tile([P, KDH, width], bf16, tag=tag, name="kh")
                    src = src_ap[:, h * KDH : (h + 1) * KDH]
                    step = KDH // npieces
                    for i in range(npieces):
                        sl = slice(i * step, (i + 1) * step)
                        eng.dma_start(t[:, sl], src[:, sl])
                    tiles.append(t)
                return tiles

            def mm1(x_tiles, ntok, w1_tiles, bias_sb, boff, gpool):
                """[P, KF, ntok] bf16 tile of gelu(w1^T x + b1)."""
                g_t = gpool.tile([P, KF, ntok], bf16, tag="g", name="g_t")
                for mi in range(KF):
                    ph = pps1.tile([P, 512], f32, tag="ph", name="ph")[:, :ntok]
                    for kd in range(KD):
                        nc.tensor.matmul(
                            ph,
                            w1_tiles[kd // KDH][:, kd % KDH, ts(mi, P)],
                            x_tiles[kd // KDH][:, kd % KDH, :],
                            start=(kd == 0),
                            stop=(kd == KD - 1),
                        )
                    nc.scalar.activation(
                        g_t[:, mi, :],
                        ph,
                        gelu,
                        bias=bias_sb[:, boff + mi : boff + mi + 1],
                    )
                return g_t

            def emit_w2_loads(src_aps, qts=None, qlo=0, qhi=4):
                """Quarter tiles [P, DQ] for each [KF, P, D] src, q-major
                interleaved across sources so arrival matches consumption."""
                if qts is None:
                    qts = [[[None] * 4 for _ in range(KF)] for _ in src_aps]
                for q in range(qlo, qhi):
                    for si, src in enumerate(src_aps):
                        for kf in range(KF):
                            t = pw2.tile([P, DQ], bf16, tag="w2", name="w2q")
                            nc.gpsimd.dma_start(t, src[kf][:, q * DQ : (q + 1) * DQ])
                            qts[si][kf][q] = t
                return qts

            ecount = [0]

            def mm2(gsrcs_list, w2qs_list, ntoks, out_drams, c0=0, c1=NDCK):
                """Token-moving second matmul over one or more token blocks.

                gsrcs_list: per block, per source: [P, KF, ntok] g tiles
                w2qs_list:  per source, [KF][4] quarter tiles [P, DQ]
                out_drams:  per block, [NG, P, 4, ntok] DRAM AP
                """
                nsrc = len(w2qs_list)
                nblk = len(gsrcs_list)
                nmm = nsrc * KF
                ys = [None] * nblk
                for c in range(c0, c1):
                    j = c % 4
                    gidx = c // 4
                    if j == 0:
                        for bi in range(nblk):
                            ys[bi] = pys.tile(
                                [P, 4, ntoks[bi]], out_dt, tag="ys", name="ys"
                            )
                    for bi in range(nblk):
                        py = pps2.tile([P, 512], f32, tag="py", name="py")[
                            :, : ntoks[bi]
                        ]
                        imm = 0
                        for si in range(nsrc):
                            for kf in range(KF):
                                nc.tensor.matmul(
                                    py,
                                    w2qs_list[si][kf][c // 10][:, ds((c % 10) * P, P)],
                                    gsrcs_list[bi][si][:, kf, :],
                                    start=(imm == 0),
                                    stop=(imm == nmm - 1),
                                )
                                imm += 1
                        dst = ys[bi][:, j, :]
                        if ecount[0] % 3 == 2:
                            nc.scalar.activation(dst, py, copyf)
                        else:
                            nc.vector.tensor_copy(out=dst, in_=py)
                        ecount[0] += 1
                    if j == 3:
                        for bi in range(nblk):
                            nc.scalar.dma_start(out_drams[bi][gidx], ys[bi])

            # ---------------- emission ----------------
            emit_biases()
            # first shared tensors kd-granular so the first matmul's inputs
            # are at the head of the queues
            sw1_t = [None, None]
            xs_t = [None, None]
            sw1_0, xs_0 = [], []
            for h in range(2):
                w1h = pw1.tile([P, KDH, FD], bf16, tag="w1", name="w1h")
                xh = pxe.tile([P, KDH, NSH], bf16, tag="xe", name="xh")
                step = KDH // 4
                for i in range(4):
                    sl = slice(i * step, (i + 1) * step)
                    nc.sync.dma_start(w1h[:, sl], sw1p[:][0][:, h * KDH :][:, sl])
                    nc.scalar.dma_start(xh[:, sl], xsp[:][0][:, h * KDH :][:, sl])
                sw1_0.append(w1h)
                xs_0.append(xh)
            sw1_t[0], xs_t[0] = sw1_0, xs_0
            xs_t[1] = load_halves(pxe, xsp[:][1], NSH, "xe", nc.sync, npieces=1)
            sw1_t[1] = load_halves(pw1, sw1p[:][1], FD, "w1", nc.scalar, npieces=1)

            w1_t = [None] * EPC
            w2q = [None] * EPC
            x_t = [None] * nslot

            def ensure_w1(e):
                if w1_t[e] is None:
                    w1_t[e] = load_halves(pw1, w1p[:][e], FD, "w1", nc.sync)

            gate_no = [0]

            def gate(gtile):
                """1-element DMA reading gtile's last kf slice: holds the
                gpsimd queue (and the w2 stream queued behind it) until that
                gelu lands, keeping early rings free for critical loads."""
                nc.gpsimd.dma_start(
                    thr[:][gate_no[0]], gtile[0:1, KF - 1 : KF, 0:1]
                )
                gate_no[0] += 1

            def load_x(k):
                x_t[k] = load_halves(pxe, xe[k][:], caps[k], "xe", nc.sync)

            ensure_w1(0)
            load_x(0)

            # shared mm1 jobs, s-major so sw1[s] is reused back-to-back;
            # w2 loads for experts 0/1 are released mid-shared-phase (jobs
            # 3/4 need no fresh input, so that bandwidth window is free)
            sw2_aps = [sw2p[:][s] for s in range(S)]
            g_sh = [[None] * S for _ in range(2)]
            for s, h in ((0, 0), (0, 1), (1, 0), (1, 1)):
                g_sh[h][s] = mm1(xs_t[h], NSH, sw1_t[s], sb1_sb, s * KF, pgs)
                if nb == 1 and (s, h) == (0, 1):
                    gate(g_sh[1][0])
                    w2q[0] = emit_w2_loads([w2p[:][0]])[0]
                elif nb == 1 and (s, h) == (1, 0):
                    gate(g_sh[0][1])
                    w2q[1] = emit_w2_loads([w2p[:][1]])[0]
            if nb == 1:
                sw2q = [[[None] * 4 for _ in range(KF)] for _ in range(S)]
            else:
                for e in range(EPC):
                    w2q[e] = emit_w2_loads([w2p[:][e]])[0]
                sw2q = emit_w2_loads(sw2_aps)

            # routed pipeline: mm1(k+1) is emitted before mm2(k) so the PE
            # never waits on the gelu tail at a block boundary
            sh_gsrcs = [[g_sh[0][0], g_sh[0][1]], [g_sh[1][0], g_sh[1][1]]]
            sh_outs = [yst[:][0], yst[:][1]]
            gt = [None] * nslot
            for k in range(nslot):
                if k + 1 < nslot:
                    ensure_w1((k + 1) // nb)
                    load_x(k + 1)
                e = k // nb
                gt[k] = mm1(x_t[k], caps[k], w1_t[e], rb1_sb, e * KF, pgr)
                # gated w2/sw2 stream, emitted in consumption order
                if nb == 1:
                    if k == 0:
                        gate(gt[0])
                        emit_w2_loads(sw2_aps, sw2q, 0, 2)
                    elif k == 1:
                        gate(gt[1])
                        w2q[2] = emit_w2_loads([w2p[:][2]])[0]
                    elif k == 2:
                        gate(gt[2])
                        w2q[3] = emit_w2_loads([w2p[:][3]])[0]
                    elif k == 3:
                        gate(gt[3])
                        emit_w2_loads(sw2_aps, sw2q, 2, 4)
                if k == nslot - 1 and nb == 1:
                    # first shared-mm2 half: DMA relief between routed mm2s
                    mm2(sh_gsrcs, sw2q, [NSH, NSH], sh_outs, 0, NDCK // 2)
                if k >= 1:
                    ep = (k - 1) // nb
                    mm2([[gt[k - 1]]], [w2q[ep]], [caps[k - 1]], [yt[k - 1][:]])
            e = (nslot - 1) // nb
            mm2([[gt[nslot - 1]]], [w2q[e]], [caps[nslot - 1]], [yt[nslot - 1][:]])
            if nb == 1:
                mm2(sh_gsrcs, sw2q, [NSH, NSH], sh_outs, NDCK // 2, NDCK)
            else:
                mm2(sh_gsrcs, sw2q, [NSH, NSH], sh_outs)

    nc.compile()
    return nc


def kernel(x, shared_w1, shared_b1, shared_w2, shared_b2,
           routed_w1, routed_b1, routed_w2, routed_b2, gate_w, gate_b):
    global LAST_EXEC_NS, LAST_MEAN_EXEC_NS, LAST_TRACE

    x = np.asarray(x, np.float32)
    x2d = np.ascontiguousarray(x.reshape(N, D))

    # ---- gating / routing (control plane) ----
    logits = x2d @ np.asarray(gate_w, np.float32) + np.asarray(gate_b, np.float32)
    logits -= logits.max(axis=-1, keepdims=True)
    probs = np.exp(logits)
    probs /= probs.sum(axis=-1, keepdims=True)                  # [N, E]
    top2 = np.argpartition(-probs, K - 1, axis=-1)[:, :K]       # [N, K]
    sel = np.zeros((N, E), np.bool_)
    sel[np.arange(N)[:, None], top2] = True

    idx_per_e = [np.nonzero(sel[:, e])[0] for e in range(E)]
    counts = np.array([len(i) for i in idx_per_e])

    # LPT assignment: experts to cores, then rank each core's experts desc
    order = np.argsort(-counts, kind="stable")
    core_exp = [[] for _ in range(NCORES)]
    core_load = np.zeros(NCORES, np.int64)
    for e in order:
        cand = min(
            (c for c in range(NCORES) if len(core_exp[c]) < EPC),
            key=lambda c: core_load[c],
        )
        core_exp[cand].append(int(e))
        core_load[cand] += counts[e]
    for c in range(NCORES):
        core_exp[c].sort(key=lambda e: -counts[e])

    # sub-blocks per expert (1 unless routing is heavily skewed)
    nb = max(1, _ceil_div(int(counts.max()), 512))
    nslot = EPC * nb
    # slot token chunks + per-rank capacities (max over cores, round to 32)
    slot_idx = [[None] * nslot for _ in range(NCORES)]
    slot_cnt = np.zeros((NCORES, nslot), np.int64)
    for c in range(NCORES):
        for r, e in enumerate(core_exp[c]):
            idx = idx_per_e[e]
            step = _ceil_div(len(idx), nb)
            for b in range(nb):
                chunk = idx[b * step : (b + 1) * step]
                slot_idx[c][r * nb + b] = chunk
                slot_cnt[c][r * nb + b] = len(chunk)
    caps = [
        max(64, int(np.ceil(slot_cnt[:, k].max() / 32)) * 32) for k in range(nslot)
    ]

    x_bf = x2d.astype(BF16)

    def pmajor_T(rows):
        """[n, D] rows -> x^T p-major [P, KD, n]."""
        return rows.T.reshape(KD, P, rows.shape[0]).transpose(1, 0, 2)

    # pre-swizzled shared weights (identical on every core)
    sw1p = np.ascontiguousarray(
        np.asarray(shared_w1).astype(BF16).reshape(S, KD, P, FD).transpose(0, 2, 1, 3)
    )
    sw2p = np.ascontiguousarray(
        np.asarray(shared_w2).astype(BF16).reshape(S, KF, P, D)
    )
    rw1_bf = np.asarray(routed_w1).astype(BF16)
    rw2_bf = np.asarray(routed_w2).astype(BF16)
    rb1_f = np.asarray(routed_b1, np.float32)
    sb1_f = np.asarray(shared_b1, np.float32)
    sb1p = np.ascontiguousarray(
        sb1_f.reshape(S, KF, P).transpose(2, 0, 1).reshape(P, S * KF)
    )

    in_maps = []
    for c in range(NCORES):
        exps = core_exp[c]
        m = {
            "w1p": np.ascontiguousarray(
                rw1_bf[exps].reshape(EPC, KD, P, FD).transpose(0, 2, 1, 3)
            ),
            "rb1": np.ascontiguousarray(
                rb1_f[exps].reshape(EPC, KF, P).transpose(2, 0, 1).reshape(P, EPC * KF)
            ),
            "w2p": np.ascontiguousarray(rw2_bf[exps].reshape(EPC, KF, P, D)),
            "xsp": np.ascontiguousarray(
                np.stack(
                    [
                        pmajor_T(x_bf[c * NS + h * NSH : c * NS + (h + 1) * NSH])
                        for h in range(2)
                    ]
                )
            ),
            "sw1p": sw1p,
            "sb1": sb1p,
            "sw2p": sw2p,
        }
        for k in range(nslot):
            arr = np.zeros((P, KD, caps[k]), BF16)
            idx = slot_idx[c][k]
            if len(idx):
                arr[:, :, : len(idx)] = pmajor_T(x_bf[idx])
            m[f"xe{k}"] = arr
        in_maps.append(m)

    nc = _build_nc(caps, nb)
    trace = bool(int(os.environ.get("MOE_TRACE", "0")))
    res = None
    for attempt in range(3):
        try:
            res = run_bass_kernel_spmd(
                nc, in_maps, core_ids=list(range(NCORES)), trace=trace
            )
            break
        except Exception:
            if attempt == 2:
                raise
    LAST_EXEC_NS = res.exec_time_ns
    LAST_MEAN_EXEC_NS = res.mean_exec_time_ns
    LAST_TRACE = res.instructions_and_trace[1] if res.instructions_and_trace else None

    # ---- combine (un-shard): transpose D-major outputs, apply combine
    # weights in fp32, scatter-add ----
    out = np.zeros((N, D), np.float32)
    for c in range(NCORES):
        ys = np.asarray(res.results[c]["yst"])          # [2, NG, P, 4, NSH]
        for h in range(2):
            out[c * NS + h * NSH : c * NS + (h + 1) * NSH] = (
                ys[h].transpose(3, 0, 2, 1).reshape(NSH, D).astype(np.float32)
            )
    for c in range(NCORES):
        for k in range(nslot):
            idx = slot_idx[c][k]
            if not len(idx):
                continue
            e = core_exp[c][k // nb]
            arr = np.asarray(res.results[c][f"yt{k}"])  # [NG, P, 4, caps[k]]
            y = (
                arr.transpose(3, 0, 2, 1)
                .reshape(caps[k], D)[: len(idx)]
                .astype(np.float32)
            )
            out[idx] += probs[idx, e][:, None] * y

    # host-side bias terms (zero in the shipped init; guarded for generality)
    shared_b2 = np.asarray(shared_b2, np.float32)
    if np.any(shared_b2):
        out += shared_b2.sum(0)[None, :]
    routed_b2 = np.asarray(routed_b2, np.float32)
    if np.any(routed_b2):
        w_full = np.where(sel, probs, 0.0).astype(np.float32)
        out += w_full @ routed_b2

    return out.reshape(B, T, D)


# revision 11
# speedup vs baseline: 1.0444x; 1.0444x over previous
"""DeepSeekMoE layer on 8 Trainium2 NeuronCores.

Strategy (expert-parallel, matching the sharding hint):
  - Host computes the (tiny) gate: softmax(x @ gate_w + gate_b), top-2
    routing, and per-expert token gather with capacity padding.  This is
    the control plane (<1% of FLOPs); all heavy matmuls run on device.
  - Each of the 8 cores owns E/8 = 4 routed experts: it receives the
    gathered tokens for those experts (transposed, bf16), the expert
    weights (bf16), and the per-(token,expert) combine weights.
  - The 2 shared experts are data-parallel over tokens: core c processes
    tokens [c*512, (c+1)*512), as two 256-token blocks that flow through
    the same pipeline as the routed blocks (with both shared experts
    accumulated in one K=2*FD matmul chain, and no combine weight).
  - Device per block: hT = w1^T @ xT (K=D in PSUM) -> Gelu(+b1) on ACT
    (hT stays FD-major) -> second matmul runs token-major: gT token-
    chunks are the stationary operand, w2 rows stream as the moving
    operand; the top-k combine weight is applied during PSUM eviction
    as a per-partition scalar multiply (tokens are partitions there)
    -> y[tok, D] staged row-wise in SBUF -> contiguous drains.
    Over-capacity experts (skewed routing) split into <=512-token
    sub-blocks that reuse the same pipeline.
  - All inputs are pre-swizzled on the host into the exact SBUF tile
    layouts so every DMA is a fully contiguous copy; DMA issue is split
    across both HWDGE rings (sync + scalar engines).
  - Host scatter-adds routed expert outputs + shared outputs back into
    token order (each token appears in exactly 2 routed lists + 1 shared
    list, so a fp32 sum reproduces the reference combine).
All matmul inputs are bf16 (PSUM accumulates fp32); biases are applied
in fp32 (b1 via the ACT bias port; b2/gate_b host-side, and they are
zero-guarded so the common all-zero case costs nothing).
"""

import os
import sys
import types

import numpy as np

# ---------------------------------------------------------------------------
# Optional NTFF trace support under axon: concourse's trace path imports
# antenv.axon_hooks, which this image lacks; shim it with the boot helper.
# ---------------------------------------------------------------------------
def _install_trace_shim():
    try:
        if "antenv.axon_hooks" in sys.modules:
            return
        from trn_agent_boot.trn_boot import _ntff_profile_via_ctypes

        hook = _ntff_profile_via_ctypes("/opt/axon/libaxon_pjrt.so")
        mod = types.ModuleType("antenv.axon_hooks")
        mod.get_axon_ntff_profile_hook = lambda: hook
        mod.set_axon_ntff_profile_hook = lambda h: None
        sys.modules["antenv.axon_hooks"] = mod
    except Exception:
        pass


_install_trace_shim()

import ml_dtypes  # noqa: E402

import concourse.bacc as bacc  # noqa: E402
import concourse.mybir as mybir  # noqa: E402
import concourse.tile as tile  # noqa: E402
import concourse.bass_utils as bass_utils  # noqa: E402
from concourse.bass import ts, ds  # noqa: E402
from concourse.bass_utils import run_bass_kernel_spmd  # noqa: E402

try:  # zero-egress sandbox: skip artifact uploads during tracing
    bass_utils.upload_artifacts = lambda tmpdir: tmpdir
except Exception:
    pass

# Problem shapes (nn_DeepSeekMoE): x [B,T,D]; E routed experts (top-K),
# S shared experts, ffn dim FD.
B, T, D = 2, 2048, 5120
FD, E, S, K = 384, 32, 2, 2
N = B * T                     # 4096 tokens
P = 128
NCORES = 8
EPC = E // NCORES             # 4 routed experts per core
NS = N // NCORES              # 512 shared-expert tokens per core
NSH = NS // 2                 # shared-block token count (256)
KD = D // P                   # 40 K-subtiles over D
KF = FD // P                  # 3 K-subtiles over FD
NDC = D // 512                # 10 output-column chunks of 512 over D

BF16 = ml_dtypes.bfloat16

LAST_EXEC_NS = None
LAST_MEAN_EXEC_NS = None
LAST_TRACE = None


def _ceil_div(a, b):
    return -(-a // b)


def _build_nc(C: int, NB: int = 1, out_dt=mybir.dt.bfloat16):
    """Build the SPMD per-core Bass program. C = per-expert token capacity.

    Input DRAM layouts are pre-swizzled on the host so that every DMA below
    is a contiguous copy:
      xep  [EPC, P, KD, C]        x tokens for expert j, transposed, p-major
      w1p  [EPC, P, KD, FD]       routed w1, p-major over D
      w2p  [EPC, NDC, P, KF, 512] routed w2, p-major over FD, chunked over D
      xsp  [2, P, KD, NSH]        shared tokens, 2 blocks, transposed, p-major
      sw1p [S, P, KD, FD]         shared w1
      sw2p [S, NDC, P, KF, 512]   shared w2
      wgt  [EPC*NB, ceil(C/P)*P]  combine weights, token-chunk-major
    Outputs are token-major: yE [EPC, C, D], ysE [NS, D].
    """
    f32 = mybir.dt.float32
    bf16 = mybir.dt.bfloat16
    nc = bacc.Bacc(None, target_bir_lowering=False)

    NEB = EPC * NB             # routed sub-blocks (<=512 tokens each)
    xep = nc.dram_tensor("xep", (NEB, P, KD, C), bf16, kind="ExternalInput")
    CP = _ceil_div(C, P) * P
    wgt = nc.dram_tensor("wgt", (NEB, CP), f32, kind="ExternalInput")
    w1p = nc.dram_tensor("w1p", (EPC, P, KD, FD), bf16, kind="ExternalInput")
    rb1 = nc.dram_tensor("rb1", (P, EPC * KF), f32, kind="ExternalInput")
    w2p = nc.dram_tensor("w2p", (EPC, NDC // 2, P, KF, 1024), bf16, kind="ExternalInput")
    xsp = nc.dram_tensor("xsp", (2, P, KD, NSH), bf16, kind="ExternalInput")
    sw1p = nc.dram_tensor("sw1p", (S, P, KD, FD), bf16, kind="ExternalInput")
    sb1 = nc.dram_tensor("sb1", (P, S * KF), f32, kind="ExternalInput")
    sw2p = nc.dram_tensor("sw2p", (S, NDC // 2, P, KF, 1024), bf16, kind="ExternalInput")
    yE = nc.dram_tensor("yE", (NEB, C, D), out_dt, kind="ExternalOutput")
    ysE = nc.dram_tensor("ysE", (NS, D), out_dt, kind="ExternalOutput")

    gelu = mybir.ActivationFunctionType.Gelu
    KDH = KD // 2              # w1 / x stream in two K-halves

    with tile.TileContext(nc) as tc:
        with (
            tc.tile_pool(name="pw1", bufs=4) as pw1,
            tc.tile_pool(name="pxe", bufs=4) as pxe,
            tc.tile_pool(name="pw2", bufs=6) as pw2,
            tc.tile_pool(name="pg", bufs=6) as pg,
            tc.tile_pool(name="pyt", bufs=4) as pyt,
            tc.tile_pool(name="pb", bufs=1) as pb,
            tc.tile_pool(name="pps1", bufs=2, space="PSUM") as pps1,
            tc.tile_pool(name="pps2", bufs=6, space="PSUM") as pps2,
        ):
            # biases -> [P, n_subtiles] with the FD subtile index on free dim
            # (emitted after the first expert's input DMAs so they don't
            # delay the first matmul in the queue)
            rb1_sb = pb.tile([P, EPC * KF], f32, tag="rb1")
            sb1_sb = pb.tile([P, S * KF], f32, tag="sb1")

            def emit_biases():
                nc.sync.dma_start(rb1_sb, rb1[:])
                nc.sync.dma_start(sb1_sb, sb1[:])

            def load_khalves(pool, src_ap, width, tag, eng, npieces=1):
                """Two [P, KD/2, width] tiles for a [P, KD, width] DRAM src."""
                tiles = []
                for h in range(2):
                    t = pool.tile([P, KDH, width], bf16, tag=tag, name="kh")
                    src = src_ap[:, h * KDH : (h + 1) * KDH]
                    step = KDH // npieces
                    for i in range(npieces):
                        eng.dma_start(
                            t[:, i * step : (i + 1) * step],
                            src[:, i * step : (i + 1) * step],
                        )
                    tiles.append(t)
                return tiles

            def mm1(x_tiles, ntok, w1_t, bias_sb, boff):
                """[P, KF, ntok] bf16 tile of gelu(w1^T x + b1)."""
                g_t = pg.tile([P, KF, ntok], bf16, tag="g", name="g_t")
                for mi in range(KF):
                    ph = pps1.tile([P, 512], f32, tag="ph", name="ph")[:, :ntok]
                    for kd in range(KD):
                        nc.tensor.matmul(
                            ph,
                            w1_t[kd // KDH][:, kd % KDH, ts(mi, P)],
                            x_tiles[kd // KDH][:, kd % KDH, :],
                            start=(kd == 0),
                            stop=(kd == KD - 1),
                        )
                    nc.scalar.activation(
                        g_t[:, mi, :],
                        ph,
                        gelu,
                        bias=bias_sb[:, boff + mi : boff + mi + 1],
                    )
                return g_t

            def mm2(gblocks, w2_list, ntoks, out_aps, scales, pre=None):
                """Token-major second matmul over one or more token blocks.

                gblocks: per block, per source: [P, KF, ntok] bf16 g tiles
                w2_list: per source, [NDC, P, KF, 512] DRAM AP (streamed once)
                out_aps: per block, [ntok, D] DRAM AP
                scales:  per block, None or fn(ci, cw) -> [cw, 1] combine-
                         weight AP applied during PSUM eviction (tokens are
                         the PSUM partitions here, so a per-partition
                         tensor_scalar multiply applies the top-k weight)
                """
                nsrc = len(w2_list)
                nmm = nsrc * KF
                drain_after = {
                    1: (0, 1024), 3: (1024, 2048), 5: (2048, 3072),
                    7: (3072, 4096), 9: (4096, 4608),
                }
                drain_final = (4608, D)
                yrows = [
                    [
                        pyt.tile([P, D], out_dt, tag="yrow", name="yrow")
                        for _ in range(_ceil_div(ntok, P))
                    ]
                    for ntok in ntoks
                ]
                w2pair = [None] * nsrc
                for mdc in range(NDC):
                    if mdc % 2 == 0:
                        if mdc == 0 and pre is not None:
                            w2pair = list(pre)
                        else:
                            for si in range(nsrc):
                                w2t = pw2.tile(
                                    [P, KF, 1024], bf16, tag="w2", name="w2t"
                                )
                                nc.scalar.dma_start(w2t, w2_list[si][mdc // 2])
                                w2pair[si] = w2t
                    off = (mdc % 2) * 512
                    w2ts = [w2pair[si][:, :, off : off + 512] for si in range(nsrc)]
                    for bi, gsrcs in enumerate(gblocks):
                        for ci in range(_ceil_div(ntoks[bi], P)):
                            cw = min(P, ntoks[bi] - ci * P)
                            py = pps2.tile(
                                [P, 512], f32, tag="py", name="py"
                            )[:cw]
                            imm = 0
                            for si in range(nsrc):
                                for kf in range(KF):
                                    nc.tensor.matmul(
                                        py,
                                        gsrcs[si][:, kf, ds(ci * P, cw)],
                                        w2ts[si][:, kf, :],
                                        start=(imm == 0),
                                        stop=(imm == nmm - 1),
                                    )
                                    imm += 1
                            dst = yrows[bi][ci][:cw, ts(mdc, 512)]
                            use_act = (mdc + ci) % 3 == 2
                            if scales[bi] is not None:
                                if use_act:
                                    nc.scalar.activation(
                                        dst, py,
                                        mybir.ActivationFunctionType.Copy,
                                        scale=scales[bi](ci, cw),
                                    )
                                else:
                                    nc.vector.tensor_scalar_mul(
                                        dst, py, scales[bi](ci, cw)
                                    )
                            elif use_act:
                                nc.scalar.activation(
                                    dst, py, mybir.ActivationFunctionType.Copy
                                )
                            else:
                                nc.vector.tensor_copy(out=dst, in_=py)
                    if mdc in drain_after:
                        lo, hi = drain_after[mdc]
                        for bi, ntok in enumerate(ntoks):
                            for ci in range(_ceil_div(ntok, P)):
                                cw = min(P, ntok - ci * P)
                                nc.gpsimd.dma_start(
                                    out_aps[bi][ds(ci * P, cw), lo:hi],
                                    yrows[bi][ci][:cw, lo:hi],
                                )
                lo, hi = drain_final
                for bi, ntok in enumerate(ntoks):
                    for ci in range(_ceil_div(ntok, P)):
                        cw = min(P, ntok - ci * P)
                        eng = nc.sync if ci % 2 == 0 else nc.scalar
                        eng.dma_start(
                            out_aps[bi][ds(ci * P, cw), lo:hi],
                            yrows[bi][ci][:cw, lo:hi],
                        )

            def load_first_block_interleaved(npieces=4):
                """Interleave the first shared block's sw1(s0)/xs(h0) piece
                DMAs across both HWDGE rings so the very first matmul's
                inputs are at the head of the queues."""
                w1_t, x_tiles = [], []
                for h in range(2):
                    w1h = pw1.tile([P, KDH, FD], bf16, tag="w1", name="w1h")
                    xh = pxe.tile([P, KDH, NSH], bf16, tag="xe", name="xh")
                    step = KDH // npieces
                    for i in range(npieces):
                        sl = slice(i * step, (i + 1) * step)
                        nc.sync.dma_start(
                            w1h[:, sl], sw1p[:][0][:, h * KDH :][:, sl]
                        )
                        nc.scalar.dma_start(
                            xh[:, sl], xsp[:][0][:, h * KDH :][:, sl]
                        )
                    w1_t.append(w1h)
                    x_tiles.append(xh)
                return w1_t, x_tiles

            # combine weights, token-chunk-major: [P, NCH] per sub-block
            NCH = _ceil_div(C, P)
            wgt_sb = pb.tile([P, NEB * NCH], f32, tag="wg")

            def routed_scale(eb):
                return lambda ci, cw: wgt_sb[:cw, eb * NCH + ci : eb * NCH + ci + 1]

            # ---------------- shared experts first (token-parallel) --------
            # The shared phase is PE-bound and DMA-light, so running it first
            # lets the DMA-bound routed phase prefetch its inputs behind the
            # shared matmuls.  h-outer / s-inner ordering; one fused mm2
            # streams each shared w2 once for all four (h, s) sub-blocks.
            sw1_t = [None, None]
            sw1_t[0], xs_h0 = load_first_block_interleaved()
            sw1_t[1] = load_khalves(pw1, sw1p[:][1], FD, "w1", nc.scalar,
                                    npieces=2)
            emit_biases()
            xs_tiles = [
                xs_h0,
                load_khalves(pxe, xsp[:][1], NSH, "xe", nc.sync, npieces=2),
            ]
            gblocks = [[None] * S for _ in range(2)]
            for s in range(S):
                for h in range(2):
                    gblocks[h][s] = mm1(xs_tiles[h], NSH, sw1_t[s], sb1_sb, s * KF)
            # shared mm2 is deferred to the very end: its w2 stream (7.9MB)
            # would otherwise collide with the routed input streams mid-kernel,
            # while at the end the PE has ~70us of matmuls and the DMA rings
            # are otherwise idle.
            shared_pending = (
                gblocks,
                [sw2p[:][s] for s in range(S)],
                [NSH, NSH],
                [ysE[:][ds(h * NSH, NSH), :] for h in range(2)],
                [None, None],
            )
            pending = None

            # ---------------- routed experts ----------------
            # software pipeline: mm1(block i+1) is emitted before mm2(block i)
            # so the PE never waits on the gelu tail or w2 prefetch at a
            # block boundary.
            for eb in range(NEB):
                e = eb // NB
                w1_t = load_khalves(pw1, w1p[:][e], FD, "w1", nc.sync)
                x_tiles = load_khalves(pxe, xep[:][eb], C, "xe", nc.sync)
                nc.sync.dma_start(
                    wgt_sb[:, eb * NCH : (eb + 1) * NCH],
                    wgt[:][eb].rearrange("(o p) -> p o", p=P),
                )
                gt = mm1(x_tiles, C, w1_t, rb1_sb, e * KF)
                if pending is not None:
                    mm2(*pending)
                pending = ([[gt]], [w2p[:][e]], [C], [yE[:][eb]],
                           [routed_scale(eb)])
            sw2_pre = []
            for s in range(S):
                w2t = pw2.tile([P, KF, 1024], bf16, tag="w2", name="w2t")
                nc.gpsimd.dma_start(w2t, sw2p[:][s][0])
                sw2_pre.append(w2t)
            mm2(*pending)
            mm2(*shared_pending, pre=sw2_pre)

    nc.compile()
    return nc


def kernel(x, shared_w1, shared_b1, shared_w2, shared_b2,
           routed_w1, routed_b1, routed_w2, routed_b2, gate_w, gate_b):
    global LAST_EXEC_NS, LAST_MEAN_EXEC_NS, LAST_TRACE

    x = np.asarray(x, np.float32)
    x2d = np.ascontiguousarray(x.reshape(N, D))

    # ---- gating / routing (control plane) ----
    logits = x2d @ np.asarray(gate_w, np.float32) + np.asarray(gate_b, np.float32)
    logits -= logits.max(axis=-1, keepdims=True)
    probs = np.exp(logits)
    probs /= probs.sum(axis=-1, keepdims=True)                  # [N, E]
    top2 = np.argpartition(-probs, K - 1, axis=-1)[:, :K]       # [N, K]
    sel = np.zeros((N, E), np.bool_)
    sel[np.arange(N)[:, None], top2] = True

    idx_per_e = [np.nonzero(sel[:, e])[0] for e in range(E)]
    counts = np.array([len(i) for i in idx_per_e])
    cmax = max(128, int(np.ceil(counts.max() / 32)) * 32)
    NB = _ceil_div(cmax, 512)        # sub-blocks per expert (1 unless skewed)
    C = max(128, int(np.ceil(cmax / NB / 32)) * 32)

    x_bf = x2d.astype(BF16)

    def pmajor_T(rows):
        """[n, D] fp32/bf16 rows -> x^T p-major [P, KD, n]."""
        return rows.T.reshape(KD, P, rows.shape[0]).transpose(1, 0, 2)

    # pre-swizzled shared weights (identical on every core)
    sw1p = np.ascontiguousarray(
        np.asarray(shared_w1).astype(BF16).reshape(S, KD, P, FD).transpose(0, 2, 1, 3)
    )
    sw2p = np.ascontiguousarray(
        np.asarray(shared_w2).astype(BF16)
        .reshape(S, KF, P, NDC // 2, 1024).transpose(0, 3, 2, 1, 4)
    )
    rw1_bf = np.asarray(routed_w1).astype(BF16)
    rw2_bf = np.asarray(routed_w2).astype(BF16)
    rb1_f = np.asarray(routed_b1, np.float32)
    sb1_f = np.asarray(shared_b1, np.float32)
    sb1p = np.ascontiguousarray(
        sb1_f.reshape(S, KF, P).transpose(2, 0, 1).reshape(P, S * KF)
    )

    CP = _ceil_div(C, P) * P
    in_maps = []
    for c in range(NCORES):
        xep = np.zeros((EPC * NB, P, KD, C), BF16)
        wgtb = np.zeros((EPC * NB, CP), np.float32)
        for j in range(EPC):
            e = c * EPC + j
            for b in range(NB):
                idx = idx_per_e[e][b * C : (b + 1) * C]
                if len(idx) == 0:
                    continue
                xep[j * NB + b, :, :, : len(idx)] = pmajor_T(x_bf[idx])
                wgtb[j * NB + b, : len(idx)] = probs[idx, e]
        xsp = np.stack(
            [
                pmajor_T(x_bf[c * NS + h * NSH : c * NS + (h + 1) * NSH])
                for h in range(2)
            ]
        )
        w1c = rw1_bf[c * EPC : (c + 1) * EPC]
        w2c = rw2_bf[c * EPC : (c + 1) * EPC]
        in_maps.append(
            {
                "xep": xep,
                "wgt": wgtb,
                "w1p": np.ascontiguousarray(
                    w1c.reshape(EPC, KD, P, FD).transpose(0, 2, 1, 3)
                ),
                "rb1": np.ascontiguousarray(
                    rb1_f[c * EPC : (c + 1) * EPC]
                    .reshape(EPC, KF, P).transpose(2, 0, 1).reshape(P, EPC * KF)
                ),
                "w2p": np.ascontiguousarray(
                    w2c.reshape(EPC, KF, P, NDC // 2, 1024).transpose(0, 3, 2, 1, 4)
                ),
                "xsp": np.ascontiguousarray(xsp),
                "sw1p": sw1p,
                "sb1": sb1p,
                "sw2p": sw2p,
            }
        )

    nc = _build_nc(C, NB)
    trace = bool(int(os.environ.get("MOE_TRACE", "0")))
    res = None
    for attempt in range(3):
        try:
            res = run_bass_kernel_spmd(
                nc, in_maps, core_ids=list(range(NCORES)), trace=trace
            )
            break
        except Exception:
            if attempt == 2:
                raise
    LAST_EXEC_NS = res.exec_time_ns
    LAST_MEAN_EXEC_NS = res.mean_exec_time_ns
    LAST_TRACE = res.instructions_and_trace[1] if res.instructions_and_trace else None

    # ---- combine (un-shard) ----
    out = np.zeros((N, D), np.float32)
    for c in range(NCORES):
        out[c * NS : (c + 1) * NS] = np.asarray(res.results[c]["ysE"], np.float32)
    for c in range(NCORES):
        yEc = res.results[c]["yE"]
        for j in range(EPC):
            e = c * EPC + j
            for b in range(NB):
                idx = idx_per_e[e][b * C : (b + 1) * C]
                if len(idx):
                    out[idx] += np.asarray(
                        yEc[j * NB + b, : len(idx), :], np.float32
                    )

    # host-side bias terms (zero in the shipped init; guarded for generality)
    shared_b2 = np.asarray(shared_b2, np.float32)
    if np.any(shared_b2):
        out += shared_b2.sum(0)[None, :]
    routed_b2 = np.asarray(routed_b2, np.float32)
    if np.any(routed_b2):
        w_full = np.where(sel, probs, 0.0).astype(np.float32)
        out += w_full @ routed_b2

    return out.reshape(B, T, D)

